# revision 23
# baseline (speedup 1.0000x reference)
"""Cross-attention kernel for Trainium2 (8 NeuronCores, data-parallel over batch).

Reference computation (per batch b):
    q = (x @ Wq.T) * gamma_q ; k = (ctx @ Wk.T) * gamma_k ; v = (ctx @ Wv.T) * gamma_v
    per head: o = softmax(q k^T / sqrt(dh)) v
    out = (concat_heads(o) @ Wo.T + bo) * gamma_out

Device strategy (per core, 4 batches, n = 4*4096 = 16384 query rows):
  - Transposed world: activations live as [channel | n]; contraction dims sit
    on partitions and softmax sums come out of the PE via a ones-column in V.
  - Heads are packed in PAIRS at partition bases {0, 64}. The two AV matmuls
    of a pair write ONE [128|512] psum tile directly (head B lands at
    partitions 64-127 via matmul tile_position=(0,64)), so no shift-DMAs.
  - Z rows (partition 40 / 104 of the pair psum) reach SBUF through the
    single pair evacuation; tiny SBUF->SBUF DMAs gather them into [4|512]
    tiles, a fast-approx DVE reciprocal per 2-pair group, then a K=4
    selector matmul broadcasts 1/Z across the 64-row halves into psum.
  - Two-deep software pipeline to keep the PE HAM-warm: chunk c emits
    q-projection(c), broadcast+normalize(c-1), out-projection(c-2), then
    attention(c). Every matmul's inputs are ready ~a chunk before it issues.
  - Output stored bf16 (halves output DMA traffic; well within tolerance).
"""

import os
import sys

import ml_dtypes
import numpy as np

BF16NP = ml_dtypes.bfloat16

for _p in ("/opt/trn_rl_repo",):
    if _p not in sys.path and os.path.isdir(_p):
        sys.path.append(_p)

import concourse.bass as bass
import concourse.mybir as mybir
import concourse.tile as tile
from concourse.bass_utils import run_bass_kernel_spmd

HEADS = 8
DH = 40
QD = 320            # query/input channel dim == inner dim
CD = 768            # context channel dim
B, NQ, NK = 32, 4096, 77
NCORES = 8
BL = B // NCORES    # batches per core = 4
NLOC = BL * NQ      # query rows per core = 16384
NKL = BL * NK       # context rows per core = 308
CHUNK = 512
NCHUNKS = NLOC // CHUNK          # 32
CHUNKS_PER_BATCH = NQ // CHUNK   # 8
NPAIR = HEADS // 2               # 4 head pairs; pair p = heads (2p, 2p+1)

F32 = mybir.dt.float32
F32R = mybir.dt.float32r
BF16 = mybir.dt.bfloat16

# K-chunking of the contraction dims
DK_Q = [(0, 128), (128, 128), (256, 64)]                       # QD = 320
DK_C = [(i * 128, 128) for i in range(6)]                      # CD = 768
JT = [(0, 128), (128, 128), (256, 64)]                         # out channels 320

LAST_EXEC_NS = None
LAST_RESULTS = None


def _split_multi_waits(nc):
    """Walrus codegen allows at most ONE semaphore wait per instruction.
    Split any instruction with N>1 waits into (N-1) same-engine NoOps, each
    carrying one wait, followed by the original instruction with the last
    wait. Engines execute their streams in order, so this is equivalent."""
    k = 0
    for blk in nc.m.functions[0].blocks:
        insts = list(blk.instructions)
        out = []
        for ins in insts:
            si = getattr(ins, "sync_info", None)
            if si is not None and len(si.on_wait) > 1:
                waits = list(si.on_wait)
                for w in waits[:-1]:
                    nop = mybir.InstNoOp(name=f"wsplit-{k}")
                    k += 1
                    nop.engine = ins.engine
                    nop.sync_info = mybir.SyncInfo(on_wait=[w], on_update=[])
                    out.append(nop)
                ins.sync_info = mybir.SyncInfo(
                    on_wait=[waits[-1]], on_update=list(si.on_update)
                )
            out.append(ins)
        if len(out) != len(insts):
            blk.instructions = out
    return nc


def _build_program():
    nc = bass.Bass(trn_type="TRN2")

    xT = nc.declare_dram_parameter("xT", [QD, NLOC], BF16, isOutput=False)
    cT = nc.declare_dram_parameter("cT", [CD, NKL], BF16, isOutput=False)
    wq = nc.declare_dram_parameter("wq", [QD, NPAIR, 104], BF16, isOutput=False)
    wk = nc.declare_dram_parameter("wk", [CD, NPAIR, 104], BF16, isOutput=False)
    wv = nc.declare_dram_parameter("wv", [CD, QD], BF16, isOutput=False)
    wo = nc.declare_dram_parameter("wo", [NPAIR, 128, QD], BF16, isOutput=False)
    bo = nc.declare_dram_parameter("bo", [QD, 1], F32, isOutput=False)
    sel = nc.declare_dram_parameter("sel", [8, NPAIR, 128], F32R, isOutput=False)
    outT = nc.declare_dram_parameter("outT", [QD, NLOC], BF16, isOutput=True)

    with tile.TileContext(nc) as tc:
        with (
            tc.tile_pool(name="consts", bufs=1) as consts,
            tc.tile_pool(name="xt", bufs=3) as xt_pool,
            tc.tile_pool(name="qt", bufs=2) as qt_pool,
            tc.tile_pool(name="ex", bufs=4) as ex_pool,
            tc.tile_pool(name="oh", bufs=3) as oh_pool,
            tc.tile_pool(name="zg", bufs=2) as zg_pool,
            tc.tile_pool(name="zf", bufs=2) as zf_pool,
            tc.tile_pool(name="zr", bufs=2) as zr_pool,
            tc.tile_pool(name="st", bufs=3) as st_pool,
            tc.tile_pool(name="oo", bufs=3) as oo_pool,
        ):
            # ---- load + stage constants ----
            def staged(shape, dtype, tag, src):
                s = consts.tile(shape, dtype, tag=f"s{tag}")
                nc.sync.dma_start(out=s, in_=src)
                t = consts.tile(shape, dtype, tag=tag)
                nc.vector.tensor_copy(out=t, in_=s)
                return t

            wq_sb = [
                staged([dk, NPAIR, 104], BF16, f"wq{i}", wq[d0 : d0 + dk, :, :])
                for i, (d0, dk) in enumerate(DK_Q)
            ]
            wo_sb = [
                staged([128, QD], BF16, f"wo{p}", wo[p, :, :]) for p in range(NPAIR)
            ]
            wk_sb = [
                staged([dk, NPAIR, 104], BF16, f"wk{i}", wk[d0 : d0 + dk, :, :])
                for i, (d0, dk) in enumerate(DK_C)
            ]
            wv_sb = [
                staged([dk, QD], BF16, f"wv{i}", wv[d0 : d0 + dk, :])
                for i, (d0, dk) in enumerate(DK_C)
            ]
            ct_sb = [
                staged([dk, NKL], BF16, f"ct{i}", cT[d0 : d0 + dk, :])
                for i, (d0, dk) in enumerate(DK_C)
            ]
            bo_sb = []
            for j, (j0, jw) in enumerate(JT):
                t = consts.tile([jw, 1], F32, tag=f"bo{j}")
                nc.sync.dma_start(out=t, in_=bo[j0 : j0 + jw, :])
                bo_sb.append(t)

            # selector matrices for the 1/Z broadcast matmul: sel_t[:, p, :]
            # is [8|128]; out rows 0-63 copy zr row 2p, rows 64-127 row 2p+1
            sel_t = consts.tile([8, NPAIR, 128], F32R, tag="sel")
            nc.sync.dma_start(out=sel_t, in_=sel[:, :, :])
            sel_sb = [sel_t[:, p, :] for p in range(NPAIR)]

            with (
                tc.tile_pool(name="ps_q", bufs=2, space="PSUM") as ps_q,
                tc.tile_pool(name="ps_sc", bufs=2, space="PSUM") as ps_sc,
                tc.tile_pool(name="ps_p1", bufs=2, space="PSUM") as ps_p1,
                tc.tile_pool(name="ps_rb", bufs=1, space="PSUM") as ps_rb,
                tc.tile_pool(name="ps_po", bufs=1, space="PSUM") as ps_po,
            ):
                # ---- setup projections (psum via the main pools) ----
                kt_sb = []
                vp_sb = []
                # kT[p]: [104 | NKL], heads of pair p at partitions 0 / 64
                for p in range(NPAIR):
                    kp = ps_q.tile([104, NKL], F32, tag="q")
                    for i in range(len(DK_C)):
                        nc.tensor.matmul(
                            kp,
                            wk_sb[i][:, p, :],
                            ct_sb[i],
                            start=(i == 0),
                            stop=(i == len(DK_C) - 1),
                        )
                    t = consts.tile([104, NKL], BF16, tag=f"kt{p}")
                    nc.scalar.copy(out=t, in_=kp)
                    kt_sb.append(t)

                # vp[b]: [77 | 8*64]; head h: cols 64h..64h+39 = v channels,
                # col 64h+40 = 1 (Z), rest 0
                for b in range(BL):
                    vb = ps_p1.tile([NK, QD], F32, tag="p1")
                    for i in range(len(DK_C)):
                        nc.tensor.matmul(
                            vb,
                            ct_sb[i][:, b * NK : (b + 1) * NK],
                            wv_sb[i],
                            start=(i == 0),
                            stop=(i == len(DK_C) - 1),
                        )
                    tf = consts.tile([NK, HEADS * 64], F32, tag=f"vpf{b}")
                    nc.vector.memset(tf, 0.0)
                    tf3 = tf.rearrange("p (h c) -> p h c", c=64)
                    vb3 = vb.rearrange("p (h c) -> p h c", c=DH)
                    nc.vector.tensor_copy(out=tf3[:, :, 0:DH], in_=vb3)
                    nc.vector.memset(tf3[:, :, DH : DH + 1], 1.0)
                    t = consts.tile([NK, HEADS * 64], BF16, tag=f"vp{b}")
                    nc.vector.tensor_copy(out=t, in_=tf)
                    vp_sb.append(t)

                # ---- pipeline stages ----
                def emit_bcst(state):
                    # broadcast 1/Z into psum and normalize: st = oh * rb
                    zr, ohs, sts = state["zr"], state["oh"], state["st"]
                    for p in range(NPAIR):
                        rb = ps_rb.tile([128, CHUNK], F32, tag="rb")
                        nc.tensor.matmul(
                            rb, sel_sb[p], zr, start=True, stop=True
                        )
                        st = st_pool.tile([128, CHUNK], BF16, tag=f"st{p}")
                        with nc.allow_low_precision(
                            reason="bf16 attention weights within tolerance"
                        ):
                            nc.vector.tensor_mul(st, ohs[p], rb)
                        sts[p] = st

                def emit_po(state):
                    n0_prev, sts_prev = state["n0"], state["st"]
                    for j, (j0, jw) in enumerate(JT):
                        po = ps_po.tile([128, CHUNK], F32, tag="po")
                        for p in range(NPAIR):
                            nc.tensor.matmul(
                                po[0:jw, :],
                                wo_sb[p][:, j0 : j0 + jw],
                                sts_prev[p],
                                start=(p == 0),
                                stop=(p == NPAIR - 1),
                            )
                        oo = oo_pool.tile([jw, CHUNK], BF16, tag="oo")
                        with nc.allow_low_precision(
                            reason="bf16 output well within tolerance"
                        ):
                            nc.vector.tensor_scalar_add(
                                out=oo, in0=po[0:jw, :], scalar1=bo_sb[j]
                            )
                        nc.sync.dma_start(
                            out=outT[j0 : j0 + jw, n0_prev : n0_prev + CHUNK], in_=oo
                        )

                def load_xt(ci):
                    n0 = ci * CHUNK
                    xts = []
                    for i, (d0, dk) in enumerate(DK_Q):
                        t = xt_pool.tile([dk, CHUNK], BF16, tag=f"xt{i}")
                        nc.sync.dma_start(out=t, in_=xT[d0 : d0 + dk, n0 : n0 + CHUNK])
                        xts.append(t)
                    return xts

                # ---- main loop over n-chunks (two-deep pipeline) ----
                pending = []        # states awaiting bcst (depth 1) / po (2)
                xt_next = load_xt(0)
                for ci in range(NCHUNKS):
                    b = ci // CHUNKS_PER_BATCH
                    n0 = ci * CHUNK
                    xts = xt_next

                    # qT pairs: [104 | CHUNK], heads at rows 0-39 / 64-103
                    qts = []
                    for p in range(NPAIR):
                        qp = ps_q.tile([104, CHUNK], F32, tag="q")
                        for i in range(len(DK_Q)):
                            nc.tensor.matmul(
                                qp,
                                wq_sb[i][:, p, :],
                                xts[i],
                                start=(i == 0),
                                stop=(i == len(DK_Q) - 1),
                            )
                        qt = qt_pool.tile([104, CHUNK], BF16, tag=f"qt{p}")
                        if p < 2:
                            nc.scalar.copy(out=qt, in_=qp)
                        else:
                            nc.vector.tensor_copy(out=qt, in_=qp)
                        qts.append(qt)

                    if ci + 1 < NCHUNKS:
                        xt_next = load_xt(ci + 1)

                    # normalize chunk ci-1; out-project chunk ci-2
                    if len(pending) >= 1:
                        emit_bcst(pending[-1])
                    if len(pending) >= 2:
                        emit_po(pending[0])
                        pending.pop(0)

                    # attention for chunk ci (per head; psum sc double-buffer)
                    state = {
                        "n0": n0,
                        "oh": [None] * NPAIR,
                        "zr": None,
                        "st": [None] * NPAIR,
                    }
                    zgt = zg_pool.tile([8, CHUNK], F32, tag="zg")
                    for p in range(NPAIR):
                        bs = b * NK
                        exs = []
                        p1 = ps_p1.tile([128, CHUNK], F32, tag="p1")
                        for half in range(2):
                            hb = 64 * half
                            sc = ps_sc.tile([NK, CHUNK], F32, tag="sc")
                            nc.tensor.matmul(
                                sc,
                                kt_sb[p][hb : hb + DH, bs : bs + NK],
                                qts[p][hb : hb + DH, :],
                                start=True,
                                stop=True,
                            )
                            ex = ex_pool.tile([NK, CHUNK], BF16, tag="ex")
                            nc.scalar.activation(
                                out=ex, in_=sc, func=mybir.ActivationFunctionType.Exp
                            )
                            exs.append(ex)
                        for half in range(2):
                            nc.tensor.matmul(
                                p1[64 * half : 64 * half + 64, :],
                                vp_sb[b][
                                    :, (2 * p + half) * 64 : (2 * p + half) * 64 + 64
                                ],
                                exs[half],
                                start=True,
                                stop=True,
                            )
                        # single evacuation of the pair tile (Z rows included)
                        oh = oh_pool.tile([128, CHUNK], F32, tag=f"oh{p}")
                        if p < 2:
                            nc.scalar.copy(out=oh, in_=p1)
                        else:
                            nc.vector.tensor_copy(out=oh, in_=p1)
                        state["oh"][p] = oh
                        # gather the pair's Z rows into the chunk tile
                        nc.sync.dma_start(
                            out=zgt[2 * p : 2 * p + 1, :],
                            in_=oh[DH : DH + 1, :],
                        )
                        nc.sync.dma_start(
                            out=zgt[2 * p + 1 : 2 * p + 2, :],
                            in_=oh[64 + DH : 64 + DH + 1, :],
                        )

                    # 1/Z on ACT: exp(-ln Z); Ln and Exp share one act table
                    lg = zf_pool.tile([8, CHUNK], F32, tag="lg")
                    nc.scalar.activation(
                        out=lg, in_=zgt, func=mybir.ActivationFunctionType.Ln
                    )
                    zr = zr_pool.tile([8, CHUNK], F32R, tag="zr")
                    with nc.allow_low_precision(
                        reason="act-table 1/Z well within tolerance"
                    ):
                        nc.scalar.activation(
                            out=zr,
                            in_=lg,
                            func=mybir.ActivationFunctionType.Exp,
                            scale=-1.0,
                        )
                    state["zr"] = zr

                    pending.append(state)

                # drain the pipeline
                emit_bcst(pending[-1])
                for stt in pending:
                    emit_po(stt)

    return _split_multi_waits(nc)


_PROGRAM = None


def _get_program():
    global _PROGRAM
    if _PROGRAM is None:
        _PROGRAM = _build_program()
    return _PROGRAM


def _prep_weights(Wq, Wk, Wv, Wo, bo, gamma_q, gamma_k, gamma_v, gamma_out):
    scale = DH ** -0.5
    Wqp = (gamma_q[:, None] * Wq) * scale          # [320i, 320d]
    Wkp = gamma_k[:, None] * Wk                    # [320i, 768d]
    Wvp = gamma_v[:, None] * Wv                    # [320i, 768d]
    Wop = gamma_out[:, None] * Wo                  # [320j, 320i]
    bop = (gamma_out * bo).astype(np.float32)[:, None]

    wq_dev = np.zeros((QD, NPAIR, 104), np.float32)
    wk_dev = np.zeros((CD, NPAIR, 104), np.float32)
    for p in range(NPAIR):
        hA, hB = 2 * p, 2 * p + 1
        wq_dev[:, p, 0:DH] = Wqp[hA * DH : (hA + 1) * DH, :].T
        wq_dev[:, p, 64 : 64 + DH] = Wqp[hB * DH : (hB + 1) * DH, :].T
        wk_dev[:, p, 0:DH] = Wkp[hA * DH : (hA + 1) * DH, :].T
        wk_dev[:, p, 64 : 64 + DH] = Wkp[hB * DH : (hB + 1) * DH, :].T
    wv_dev = np.ascontiguousarray(Wvp.T, dtype=np.float32)     # [768, 320]
    # st rows per pair: 0..39 = head A channels, 40 = junk (Z/Z), 64..103 =
    # head B channels, 104 = junk; the rest is zero
    wo_dev = np.zeros((NPAIR, 128, QD), np.float32)
    for p in range(NPAIR):
        hA, hB = 2 * p, 2 * p + 1
        wo_dev[p, 0:DH, :] = Wop[:, hA * DH : (hA + 1) * DH].T
        wo_dev[p, 64 : 64 + DH, :] = Wop[:, hB * DH : (hB + 1) * DH].T
    sel_dev = np.zeros((8, NPAIR, 128), np.float32)
    for p in range(NPAIR):
        sel_dev[2 * p, p, 0:64] = 1.0
        sel_dev[2 * p + 1, p, 64:128] = 1.0
    return wq_dev, wk_dev, wv_dev, wo_dev, bop, sel_dev


def kernel(x, context, Wq, Wk, Wv, Wo, bo, gamma_q, gamma_k, gamma_v, gamma_out):
    global LAST_EXEC_NS, LAST_RESULTS
    x = np.asarray(x, np.float32)
    context = np.asarray(context, np.float32)
    wq_dev, wk_dev, wv_dev, wo_dev, bop, sel_dev = _prep_weights(
        np.asarray(Wq, np.float32), np.asarray(Wk, np.float32),
        np.asarray(Wv, np.float32), np.asarray(Wo, np.float32),
        np.asarray(bo, np.float32), np.asarray(gamma_q, np.float32),
        np.asarray(gamma_k, np.float32), np.asarray(gamma_v, np.float32),
        np.asarray(gamma_out, np.float32),
    )

    in_maps = []
    for c in range(NCORES):
        xs = x[c * BL : (c + 1) * BL].reshape(NLOC, QD)
        cs = context[c * BL : (c + 1) * BL].reshape(NKL, CD)
        in_maps.append(
            {
                "xT": np.ascontiguousarray(xs.T).astype(BF16NP),
                "cT": np.ascontiguousarray(cs.T).astype(BF16NP),
                "wq": wq_dev.astype(BF16NP),
                "wk": wk_dev.astype(BF16NP),
                "wv": wv_dev.astype(BF16NP),
                "wo": wo_dev.astype(BF16NP),
                "bo": bop,
                "sel": sel_dev,
            }
        )

    nc = _get_program()
    res = run_bass_kernel_spmd(nc, in_maps, list(range(NCORES)))
    LAST_EXEC_NS = res.exec_time_ns
    LAST_RESULTS = res

    out = np.empty((B, NQ, QD), np.float32)
    for c in range(NCORES):
        out[c * BL : (c + 1) * BL] = (
            np.asarray(res.results[c]["outT"]).astype(np.float32).T.reshape(BL, NQ, QD)
        )
    return out


# revision 27
# speedup vs baseline: 1.1162x; 1.1162x over previous
"""Cross-attention kernel for Trainium2 (8 NeuronCores, data-parallel over batch).

Reference computation (per batch b):
    q = (x @ Wq.T) * gamma_q ; k = (ctx @ Wk.T) * gamma_k ; v = (ctx @ Wv.T) * gamma_v
    per head: o = softmax(q k^T / sqrt(dh)) v
    out = (concat_heads(o) @ Wo.T + bo) * gamma_out

Device strategy (per core, 4 batches, n = 4*4096 = 16384 query rows):
  - Transposed world: activations live as [channel | n]; contraction dims sit
    on partitions and softmax sums come out of the PE via a ones-column in V.
  - Heads are packed in PAIRS at partition bases {0, 64}. The two AV matmuls
    of a pair write ONE [128|512] psum tile directly (head B lands at
    partitions 64-127 via matmul tile_position=(0,64)), so no shift-DMAs.
  - Z rows (partition 40 / 104 of the pair psum) reach SBUF through the
    single pair evacuation; tiny SBUF->SBUF DMAs gather them into [4|512]
    tiles, a fast-approx DVE reciprocal per 2-pair group, then a K=4
    selector matmul broadcasts 1/Z across the 64-row halves into psum.
  - Two-deep software pipeline to keep the PE HAM-warm: chunk c emits
    q-projection(c), broadcast+normalize(c-1), out-projection(c-2), then
    attention(c). Every matmul's inputs are ready ~a chunk before it issues.
  - Output stored bf16 (halves output DMA traffic; well within tolerance).
"""

import os
import sys

import ml_dtypes
import numpy as np

BF16NP = ml_dtypes.bfloat16

for _p in ("/opt/trn_rl_repo",):
    if _p not in sys.path and os.path.isdir(_p):
        sys.path.append(_p)

import concourse.bass as bass
import concourse.mybir as mybir
import concourse.tile as tile
from concourse.bass_utils import run_bass_kernel_spmd

HEADS = 8
DH = 40
QD = 320            # query/input channel dim == inner dim
CD = 768            # context channel dim
B, NQ, NK = 32, 4096, 77
NCORES = 8
BL = B // NCORES    # batches per core = 4
NLOC = BL * NQ      # query rows per core = 16384
NKL = BL * NK       # context rows per core = 308
CHUNK = 512
NCHUNKS = NLOC // CHUNK          # 32
CHUNKS_PER_BATCH = NQ // CHUNK   # 8
NPAIR = HEADS // 2               # 4 head pairs; pair p = heads (2p, 2p+1)

F32 = mybir.dt.float32
F32R = mybir.dt.float32r
BF16 = mybir.dt.bfloat16

# K-chunking of the contraction dims
DK_Q = [(0, 128), (128, 128), (256, 64)]                       # QD = 320
DK_C = [(i * 128, 128) for i in range(6)]                      # CD = 768
JT = [(0, 128), (128, 128), (256, 64)]                         # out channels 320

LAST_EXEC_NS = None
LAST_RESULTS = None


def _split_multi_waits(nc):
    """Walrus codegen allows at most ONE semaphore wait per instruction.
    Split any instruction with N>1 waits into (N-1) same-engine NoOps, each
    carrying one wait, followed by the original instruction with the last
    wait. Engines execute their streams in order, so this is equivalent."""
    k = 0
    for blk in nc.m.functions[0].blocks:
        insts = list(blk.instructions)
        out = []
        for ins in insts:
            si = getattr(ins, "sync_info", None)
            if si is not None and len(si.on_wait) > 1:
                waits = list(si.on_wait)
                for w in waits[:-1]:
                    nop = mybir.InstNoOp(name=f"wsplit-{k}")
                    k += 1
                    nop.engine = ins.engine
                    nop.sync_info = mybir.SyncInfo(on_wait=[w], on_update=[])
                    out.append(nop)
                ins.sync_info = mybir.SyncInfo(
                    on_wait=[waits[-1]], on_update=list(si.on_update)
                )
            out.append(ins)
        if len(out) != len(insts):
            blk.instructions = out
    return nc


def _build_program():
    nc = bass.Bass(trn_type="TRN2")

    xT = nc.declare_dram_parameter("xT", [QD, NLOC], BF16, isOutput=False)
    cT = nc.declare_dram_parameter("cT", [CD, NKL], BF16, isOutput=False)
    wq = nc.declare_dram_parameter("wq", [QD, NPAIR, 104], BF16, isOutput=False)
    wk = nc.declare_dram_parameter("wk", [CD, NPAIR, 104], BF16, isOutput=False)
    wv = nc.declare_dram_parameter("wv", [CD, QD], BF16, isOutput=False)
    wo = nc.declare_dram_parameter("wo", [NPAIR, 128, QD], BF16, isOutput=False)
    bo = nc.declare_dram_parameter("bo", [QD, 1], F32, isOutput=False)
    sel = nc.declare_dram_parameter("sel", [8, NPAIR, 128], F32R, isOutput=False)
    outT = nc.declare_dram_parameter("outT", [QD, NLOC], BF16, isOutput=True)

    with tile.TileContext(nc) as tc:
        with (
            tc.tile_pool(name="consts", bufs=1) as consts,
            tc.tile_pool(name="xt", bufs=3) as xt_pool,
            tc.tile_pool(name="qt", bufs=3) as qt_pool,
            tc.tile_pool(name="ex", bufs=3) as ex_pool,
            tc.tile_pool(name="oh", bufs=3) as oh_pool,
            tc.tile_pool(name="zg", bufs=2) as zg_pool,
            tc.tile_pool(name="zf", bufs=2) as zf_pool,
            tc.tile_pool(name="zr", bufs=3) as zr_pool,
            tc.tile_pool(name="st", bufs=3) as st_pool,
            tc.tile_pool(name="oo", bufs=3) as oo_pool,
        ):
            # ---- load + stage constants ----
            def staged(shape, dtype, tag, src):
                s = consts.tile(shape, dtype, tag=f"s{tag}")
                nc.sync.dma_start(out=s, in_=src)
                t = consts.tile(shape, dtype, tag=tag)
                nc.vector.tensor_copy(out=t, in_=s)
                return t

            wq_sb = [
                staged([dk, NPAIR, 104], BF16, f"wq{i}", wq[d0 : d0 + dk, :, :])
                for i, (d0, dk) in enumerate(DK_Q)
            ]
            wo_sb = [
                staged([128, QD], BF16, f"wo{p}", wo[p, :, :]) for p in range(NPAIR)
            ]
            wk_sb = [
                staged([dk, NPAIR, 104], BF16, f"wk{i}", wk[d0 : d0 + dk, :, :])
                for i, (d0, dk) in enumerate(DK_C)
            ]
            wv_sb = [
                staged([dk, QD], BF16, f"wv{i}", wv[d0 : d0 + dk, :])
                for i, (d0, dk) in enumerate(DK_C)
            ]
            ct_sb = [
                staged([dk, NKL], BF16, f"ct{i}", cT[d0 : d0 + dk, :])
                for i, (d0, dk) in enumerate(DK_C)
            ]
            bo_sb = []
            for j, (j0, jw) in enumerate(JT):
                t = consts.tile([jw, 1], F32, tag=f"bo{j}")
                nc.sync.dma_start(out=t, in_=bo[j0 : j0 + jw, :])
                bo_sb.append(t)

            # selector matrices for the 1/Z broadcast matmul: sel_t[:, p, :]
            # is [8|128]; out rows 0-63 copy zr row 2p, rows 64-127 row 2p+1
            sel_t = consts.tile([8, NPAIR, 128], F32R, tag="sel")
            nc.sync.dma_start(out=sel_t, in_=sel[:, :, :])
            sel_sb = [sel_t[:, p, :] for p in range(NPAIR)]

            with (
                tc.tile_pool(name="ps_q", bufs=2, space="PSUM") as ps_q,
                tc.tile_pool(name="ps_sc", bufs=2, space="PSUM") as ps_sc,
                tc.tile_pool(name="ps_p1", bufs=2, space="PSUM") as ps_p1,
                tc.tile_pool(name="ps_rb", bufs=1, space="PSUM") as ps_rb,
                tc.tile_pool(name="ps_po", bufs=1, space="PSUM") as ps_po,
            ):
                # ---- setup projections (psum via the main pools) ----
                kt_sb = []
                vp_sb = []
                # kT[p]: [104 | NKL], heads of pair p at partitions 0 / 64
                for p in range(NPAIR):
                    kp = ps_q.tile([104, NKL], F32, tag="q")
                    for i in range(len(DK_C)):
                        nc.tensor.matmul(
                            kp,
                            wk_sb[i][:, p, :],
                            ct_sb[i],
                            start=(i == 0),
                            stop=(i == len(DK_C) - 1),
                        )
                    t = consts.tile([104, NKL], BF16, tag=f"kt{p}")
                    nc.scalar.copy(out=t, in_=kp)
                    kt_sb.append(t)

                # vp[b]: [77 | 8*64]; head h: cols 64h..64h+39 = v channels,
                # col 64h+40 = 1 (Z), rest 0
                for b in range(BL):
                    vb = ps_p1.tile([NK, QD], F32, tag="p1")
                    for i in range(len(DK_C)):
                        nc.tensor.matmul(
                            vb,
                            ct_sb[i][:, b * NK : (b + 1) * NK],
                            wv_sb[i],
                            start=(i == 0),
                            stop=(i == len(DK_C) - 1),
                        )
                    tf = consts.tile([NK, HEADS * 64], F32, tag=f"vpf{b}")
                    nc.vector.memset(tf, 0.0)
                    tf3 = tf.rearrange("p (h c) -> p h c", c=64)
                    vb3 = vb.rearrange("p (h c) -> p h c", c=DH)
                    nc.vector.tensor_copy(out=tf3[:, :, 0:DH], in_=vb3)
                    nc.vector.memset(tf3[:, :, DH : DH + 1], 1.0)
                    t = consts.tile([NK, HEADS * 64], BF16, tag=f"vp{b}")
                    nc.vector.tensor_copy(out=t, in_=tf)
                    vp_sb.append(t)

                # ---- pipeline stages ----
                def emit_bcst(state):
                    # broadcast 1/Z into psum and normalize: st = oh * rb
                    zr, ohs = state["zr"], state["oh"]
                    sts = state["st"] = [None] * NPAIR
                    for p in range(NPAIR):
                        rb = ps_rb.tile([128, CHUNK], F32, tag="rb")
                        nc.tensor.matmul(
                            rb, sel_sb[p], zr, start=True, stop=True
                        )
                        st = st_pool.tile([128, CHUNK], BF16, tag=f"st{p}")
                        with nc.allow_low_precision(
                            reason="bf16 attention weights within tolerance"
                        ):
                            nc.vector.tensor_mul(st, ohs[p], rb)
                        sts[p] = st

                def emit_po(state):
                    n0_prev, sts_prev = state["n0"], state["st"]
                    for j, (j0, jw) in enumerate(JT):
                        po = ps_po.tile([128, CHUNK], F32, tag="po")
                        for p in range(NPAIR):
                            nc.tensor.matmul(
                                po[0:jw, :],
                                wo_sb[p][:, j0 : j0 + jw],
                                sts_prev[p],
                                start=(p == 0),
                                stop=(p == NPAIR - 1),
                            )
                        oo = oo_pool.tile([jw, CHUNK], BF16, tag="oo")
                        with nc.allow_low_precision(
                            reason="bf16 output well within tolerance"
                        ):
                            nc.vector.tensor_scalar_add(
                                out=oo, in0=po[0:jw, :], scalar1=bo_sb[j]
                            )
                        nc.sync.dma_start(
                            out=outT[j0 : j0 + jw, n0_prev : n0_prev + CHUNK], in_=oo
                        )

                # ---- pipeline stages, each consuming the previous chunk's
                # products so PE matmuls are (transitively) wait-free ----
                states = [None] * NCHUNKS

                def stage_x(ci):
                    # prefetch x chunk
                    n0 = ci * CHUNK
                    xts = []
                    for i, (d0, dk) in enumerate(DK_Q):
                        t = xt_pool.tile([dk, CHUNK], BF16, tag=f"xt{i}")
                        nc.sync.dma_start(out=t, in_=xT[d0 : d0 + dk, n0 : n0 + CHUNK])
                        xts.append(t)
                    states[ci] = {"xt": xts, "n0": n0, "b": ci // CHUNKS_PER_BATCH}

                def stage_q(ci):
                    # q projection: [104 | CHUNK] pairs, heads at rows 0/64
                    state = states[ci]
                    qts = []
                    for p in range(NPAIR):
                        qp = ps_q.tile([104, CHUNK], F32, tag="q")
                        for i in range(len(DK_Q)):
                            nc.tensor.matmul(
                                qp,
                                wq_sb[i][:, p, :],
                                state["xt"][i],
                                start=(i == 0),
                                stop=(i == len(DK_Q) - 1),
                            )
                        qt = qt_pool.tile([104, CHUNK], BF16, tag=f"qt{p}")
                        if p < 2:
                            nc.scalar.copy(out=qt, in_=qp)
                        else:
                            nc.vector.tensor_copy(out=qt, in_=qp)
                        qts.append(qt)
                    state["qt"] = qts

                def stage_s(ci):
                    # scores + exp per head
                    state = states[ci]
                    bs = state["b"] * NK
                    exs = []
                    for p in range(NPAIR):
                        for half in range(2):
                            hb = 64 * half
                            sc = ps_sc.tile([NK, CHUNK], F32, tag="sc")
                            nc.tensor.matmul(
                                sc,
                                kt_sb[p][hb : hb + DH, bs : bs + NK],
                                state["qt"][p][hb : hb + DH, :],
                                start=True,
                                stop=True,
                            )
                            ex = ex_pool.tile([NK, CHUNK], BF16, tag=f"ex{2 * p + half}")
                            nc.scalar.activation(
                                out=ex, in_=sc, func=mybir.ActivationFunctionType.Exp
                            )
                            exs.append(ex)
                    state["ex"] = exs

                def stage_v(ci):
                    # attention-value matmuls, pair evacuation, Z chain
                    state = states[ci]
                    b = state["b"]
                    state["oh"] = [None] * NPAIR
                    zgt = zg_pool.tile([8, CHUNK], F32, tag="zg")
                    for p in range(NPAIR):
                        p1 = ps_p1.tile([128, CHUNK], F32, tag="p1")
                        for half in range(2):
                            nc.tensor.matmul(
                                p1[64 * half : 64 * half + 64, :],
                                vp_sb[b][
                                    :, (2 * p + half) * 64 : (2 * p + half) * 64 + 64
                                ],
                                state["ex"][2 * p + half],
                                start=True,
                                stop=True,
                            )
                        # single evacuation of the pair tile (Z rows included)
                        oh = oh_pool.tile([128, CHUNK], F32, tag=f"oh{p}")
                        if p < 2:
                            nc.scalar.copy(out=oh, in_=p1)
                        else:
                            nc.vector.tensor_copy(out=oh, in_=p1)
                        state["oh"][p] = oh
                        # gather the pair's Z rows into the chunk tile
                        nc.sync.dma_start(
                            out=zgt[2 * p : 2 * p + 1, :],
                            in_=oh[DH : DH + 1, :],
                        )
                        nc.sync.dma_start(
                            out=zgt[2 * p + 1 : 2 * p + 2, :],
                            in_=oh[64 + DH : 64 + DH + 1, :],
                        )
                    # 1/Z on ACT: exp(-ln Z); Ln and Exp share one act table
                    lg = zf_pool.tile([8, CHUNK], F32, tag="lg")
                    nc.scalar.activation(
                        out=lg, in_=zgt, func=mybir.ActivationFunctionType.Ln
                    )
                    zr = zr_pool.tile([8, CHUNK], F32R, tag="zr")
                    with nc.allow_low_precision(
                        reason="act-table 1/Z well within tolerance"
                    ):
                        nc.scalar.activation(
                            out=zr,
                            in_=lg,
                            func=mybir.ActivationFunctionType.Exp,
                            scale=-1.0,
                        )
                    state["zr"] = zr

                # ---- main loop: stages offset so inputs are a chunk old ----
                for ci in range(NCHUNKS + 4):
                    if ci < NCHUNKS:
                        if ci == 0:
                            stage_x(0)
                        stage_q(ci)
                        if ci + 1 < NCHUNKS:
                            stage_x(ci + 1)
                    if 1 <= ci < NCHUNKS + 1:
                        stage_s(ci - 1)
                    if 2 <= ci < NCHUNKS + 2:
                        stage_v(ci - 2)
                    if 3 <= ci < NCHUNKS + 3:
                        emit_bcst(states[ci - 3])
                    if 4 <= ci < NCHUNKS + 4:
                        emit_po(states[ci - 4])
                        states[ci - 4] = None

    return _split_multi_waits(nc)


_PROGRAM = None


def _get_program():
    global _PROGRAM
    if _PROGRAM is None:
        _PROGRAM = _build_program()
    return _PROGRAM


def _prep_weights(Wq, Wk, Wv, Wo, bo, gamma_q, gamma_k, gamma_v, gamma_out):
    scale = DH ** -0.5
    Wqp = (gamma_q[:, None] * Wq) * scale          # [320i, 320d]
    Wkp = gamma_k[:, None] * Wk                    # [320i, 768d]
    Wvp = gamma_v[:, None] * Wv                    # [320i, 768d]
    Wop = gamma_out[:, None] * Wo                  # [320j, 320i]
    bop = (gamma_out * bo).astype(np.float32)[:, None]

    wq_dev = np.zeros((QD, NPAIR, 104), np.float32)
    wk_dev = np.zeros((CD, NPAIR, 104), np.float32)
    for p in range(NPAIR):
        hA, hB = 2 * p, 2 * p + 1
        wq_dev[:, p, 0:DH] = Wqp[hA * DH : (hA + 1) * DH, :].T
        wq_dev[:, p, 64 : 64 + DH] = Wqp[hB * DH : (hB + 1) * DH, :].T
        wk_dev[:, p, 0:DH] = Wkp[hA * DH : (hA + 1) * DH, :].T
        wk_dev[:, p, 64 : 64 + DH] = Wkp[hB * DH : (hB + 1) * DH, :].T
    wv_dev = np.ascontiguousarray(Wvp.T, dtype=np.float32)     # [768, 320]
    # st rows per pair: 0..39 = head A channels, 40 = junk (Z/Z), 64..103 =
    # head B channels, 104 = junk; the rest is zero
    wo_dev = np.zeros((NPAIR, 128, QD), np.float32)
    for p in range(NPAIR):
        hA, hB = 2 * p, 2 * p + 1
        wo_dev[p, 0:DH, :] = Wop[:, hA * DH : (hA + 1) * DH].T
        wo_dev[p, 64 : 64 + DH, :] = Wop[:, hB * DH : (hB + 1) * DH].T
    sel_dev = np.zeros((8, NPAIR, 128), np.float32)
    for p in range(NPAIR):
        sel_dev[2 * p, p, 0:64] = 1.0
        sel_dev[2 * p + 1, p, 64:128] = 1.0
    return wq_dev, wk_dev, wv_dev, wo_dev, bop, sel_dev


def kernel(x, context, Wq, Wk, Wv, Wo, bo, gamma_q, gamma_k, gamma_v, gamma_out):
    global LAST_EXEC_NS, LAST_RESULTS
    x = np.asarray(x, np.float32)
    context = np.asarray(context, np.float32)
    wq_dev, wk_dev, wv_dev, wo_dev, bop, sel_dev = _prep_weights(
        np.asarray(Wq, np.float32), np.asarray(Wk, np.float32),
        np.asarray(Wv, np.float32), np.asarray(Wo, np.float32),
        np.asarray(bo, np.float32), np.asarray(gamma_q, np.float32),
        np.asarray(gamma_k, np.float32), np.asarray(gamma_v, np.float32),
        np.asarray(gamma_out, np.float32),
    )

    in_maps = []
    for c in range(NCORES):
        xs = x[c * BL : (c + 1) * BL].reshape(NLOC, QD)
        cs = context[c * BL : (c + 1) * BL].reshape(NKL, CD)
        in_maps.append(
            {
                "xT": np.ascontiguousarray(xs.T).astype(BF16NP),
                "cT": np.ascontiguousarray(cs.T).astype(BF16NP),
                "wq": wq_dev.astype(BF16NP),
                "wk": wk_dev.astype(BF16NP),
                "wv": wv_dev.astype(BF16NP),
                "wo": wo_dev.astype(BF16NP),
                "bo": bop,
                "sel": sel_dev,
            }
        )

    nc = _get_program()
    res = run_bass_kernel_spmd(nc, in_maps, list(range(NCORES)))
    LAST_EXEC_NS = res.exec_time_ns
    LAST_RESULTS = res

    out = np.empty((B, NQ, QD), np.float32)
    for c in range(NCORES):
        out[c * BL : (c + 1) * BL] = (
            np.asarray(res.results[c]["outT"]).astype(np.float32).T.reshape(BL, NQ, QD)
        )
    return out


# revision 29
# speedup vs baseline: 1.4116x; 1.2647x over previous
"""Cross-attention kernel for Trainium2 (8 NeuronCores, data-parallel over batch).

Reference computation (per batch b):
    q = (x @ Wq.T) * gamma_q ; k = (ctx @ Wk.T) * gamma_k ; v = (ctx @ Wv.T) * gamma_v
    per head: o = softmax(q k^T / sqrt(dh)) v
    out = (concat_heads(o) @ Wo.T + bo) * gamma_out

Device strategy (per core, 4 batches, n = 4*4096 = 16384 query rows):
  - Transposed world: activations live as [channel | n]; contraction dims sit
    on partitions and softmax sums come out of the PE via a ones-column in V.
  - Heads are packed in PAIRS at partition bases {0, 64}. The two AV matmuls
    of a pair write ONE [128|512] psum tile directly (head B lands at
    partitions 64-127 via matmul tile_position=(0,64)), so no shift-DMAs.
  - Z rows (partition 40 / 104 of the pair psum) reach SBUF through the
    single pair evacuation; tiny SBUF->SBUF DMAs gather them into [4|512]
    tiles, a fast-approx DVE reciprocal per 2-pair group, then a K=4
    selector matmul broadcasts 1/Z across the 64-row halves into psum.
  - Two-deep software pipeline to keep the PE HAM-warm: chunk c emits
    q-projection(c), broadcast+normalize(c-1), out-projection(c-2), then
    attention(c). Every matmul's inputs are ready ~a chunk before it issues.
  - Output stored bf16 (halves output DMA traffic; well within tolerance).
"""

import os
import sys

import ml_dtypes
import numpy as np

BF16NP = ml_dtypes.bfloat16

for _p in ("/opt/trn_rl_repo",):
    if _p not in sys.path and os.path.isdir(_p):
        sys.path.append(_p)

import concourse.bass as bass
import concourse.mybir as mybir
import concourse.tile as tile
from concourse.bass_utils import run_bass_kernel_spmd

HEADS = 8
DH = 40
QD = 320            # query/input channel dim == inner dim
CD = 768            # context channel dim
B, NQ, NK = 32, 4096, 77
NCORES = 8
BL = B // NCORES    # batches per core = 4
NLOC = BL * NQ      # query rows per core = 16384
NKL = BL * NK       # context rows per core = 308
CHUNK = 512
NCHUNKS = NLOC // CHUNK          # 32
CHUNKS_PER_BATCH = NQ // CHUNK   # 8
NPAIR = HEADS // 2               # 4 head pairs; pair p = heads (2p, 2p+1)

F32 = mybir.dt.float32
F32R = mybir.dt.float32r
BF16 = mybir.dt.bfloat16

# K-chunking of the contraction dims
DK_Q = [(0, 128), (128, 128), (256, 64)]                       # QD = 320
DK_C = [(i * 128, 128) for i in range(6)]                      # CD = 768
JT = [(0, 128), (128, 128), (256, 64)]                         # out channels 320

LAST_EXEC_NS = None
LAST_RESULTS = None


def _split_multi_waits(nc):
    """Walrus codegen allows at most ONE semaphore wait per instruction.
    Split any instruction with N>1 waits into (N-1) same-engine NoOps, each
    carrying one wait, followed by the original instruction with the last
    wait. Engines execute their streams in order, so this is equivalent."""
    k = 0
    for blk in nc.m.functions[0].blocks:
        insts = list(blk.instructions)
        out = []
        for ins in insts:
            si = getattr(ins, "sync_info", None)
            if si is not None and len(si.on_wait) > 1:
                waits = list(si.on_wait)
                for w in waits[:-1]:
                    nop = mybir.InstNoOp(name=f"wsplit-{k}")
                    k += 1
                    nop.engine = ins.engine
                    nop.sync_info = mybir.SyncInfo(on_wait=[w], on_update=[])
                    out.append(nop)
                ins.sync_info = mybir.SyncInfo(
                    on_wait=[waits[-1]], on_update=list(si.on_update)
                )
            out.append(ins)
        if len(out) != len(insts):
            blk.instructions = out
    return nc


def _build_program():
    nc = bass.Bass(trn_type="TRN2")

    xT = nc.declare_dram_parameter("xT", [QD, NLOC], BF16, isOutput=False)
    cT = nc.declare_dram_parameter("cT", [CD, NKL], BF16, isOutput=False)
    wq = nc.declare_dram_parameter("wq", [QD, NPAIR, 104], BF16, isOutput=False)
    wk = nc.declare_dram_parameter("wk", [CD, NPAIR, 104], BF16, isOutput=False)
    wv = nc.declare_dram_parameter("wv", [CD, QD], BF16, isOutput=False)
    wo = nc.declare_dram_parameter("wo", [NPAIR, 128, QD], BF16, isOutput=False)
    bo = nc.declare_dram_parameter("bo", [QD, 1], F32, isOutput=False)
    sel = nc.declare_dram_parameter("sel", [8, NPAIR, 128], F32R, isOutput=False)
    outT = nc.declare_dram_parameter("outT", [QD, NLOC], BF16, isOutput=True)

    with tile.TileContext(nc) as tc:
        with (
            tc.tile_pool(name="consts", bufs=1) as consts,
            tc.tile_pool(name="xt", bufs=3) as xt_pool,
            tc.tile_pool(name="qt", bufs=3) as qt_pool,
            tc.tile_pool(name="ex", bufs=3) as ex_pool,
            tc.tile_pool(name="oh", bufs=3) as oh_pool,
            tc.tile_pool(name="zg", bufs=2) as zg_pool,
            tc.tile_pool(name="zf", bufs=2) as zf_pool,
            tc.tile_pool(name="zr", bufs=3) as zr_pool,
            tc.tile_pool(name="st", bufs=3) as st_pool,
            tc.tile_pool(name="oo", bufs=3) as oo_pool,
        ):
            # ---- load + stage constants ----
            def staged(shape, dtype, tag, src):
                s = consts.tile(shape, dtype, tag=f"s{tag}")
                nc.sync.dma_start(out=s, in_=src)
                t = consts.tile(shape, dtype, tag=tag)
                nc.vector.tensor_copy(out=t, in_=s)
                return t

            wq_sb = [
                staged([dk, NPAIR, 104], BF16, f"wq{i}", wq[d0 : d0 + dk, :, :])
                for i, (d0, dk) in enumerate(DK_Q)
            ]
            wo_sb = [
                staged([128, QD], BF16, f"wo{p}", wo[p, :, :]) for p in range(NPAIR)
            ]
            wk_sb = [
                staged([dk, NPAIR, 104], BF16, f"wk{i}", wk[d0 : d0 + dk, :, :])
                for i, (d0, dk) in enumerate(DK_C)
            ]
            wv_sb = [
                staged([dk, QD], BF16, f"wv{i}", wv[d0 : d0 + dk, :])
                for i, (d0, dk) in enumerate(DK_C)
            ]
            ct_sb = [
                staged([dk, NKL], BF16, f"ct{i}", cT[d0 : d0 + dk, :])
                for i, (d0, dk) in enumerate(DK_C)
            ]
            bo_sb = []
            for j, (j0, jw) in enumerate(JT):
                t = consts.tile([jw, 1], F32, tag=f"bo{j}")
                nc.sync.dma_start(out=t, in_=bo[j0 : j0 + jw, :])
                bo_sb.append(t)

            # selector matrices for the 1/Z broadcast matmul: sel_t[:, p, :]
            # is [8|128]; out rows 0-63 copy zr row 2p, rows 64-127 row 2p+1
            sel_t = consts.tile([8, NPAIR, 128], F32R, tag="sel")
            nc.sync.dma_start(out=sel_t, in_=sel[:, :, :])
            sel_sb = [sel_t[:, p, :] for p in range(NPAIR)]

            with (
                tc.tile_pool(name="ps_q", bufs=2, space="PSUM") as ps_q,
                tc.tile_pool(name="ps_sc", bufs=2, space="PSUM") as ps_sc,
                tc.tile_pool(name="ps_p1", bufs=2, space="PSUM") as ps_p1,
                tc.tile_pool(name="ps_rb", bufs=1, space="PSUM") as ps_rb,
                tc.tile_pool(name="ps_po", bufs=1, space="PSUM") as ps_po,
            ):
                # ---- setup projections (psum via the main pools) ----
                kt_sb = []
                vp_sb = []
                # kT[p]: [104 | NKL], heads of pair p at partitions 0 / 64
                for p in range(NPAIR):
                    kp = ps_q.tile([104, NKL], F32, tag="q")
                    for i in range(len(DK_C)):
                        nc.tensor.matmul(
                            kp,
                            wk_sb[i][:, p, :],
                            ct_sb[i],
                            start=(i == 0),
                            stop=(i == len(DK_C) - 1),
                        )
                    t = consts.tile([104, NKL], BF16, tag=f"kt{p}")
                    nc.scalar.copy(out=t, in_=kp)
                    kt_sb.append(t)

                # vp[b]: [77 | 8*64]; head h: cols 64h..64h+39 = v channels,
                # col 64h+40 = 1 (Z), rest 0
                for b in range(BL):
                    vb = ps_p1.tile([NK, QD], F32, tag="p1")
                    for i in range(len(DK_C)):
                        nc.tensor.matmul(
                            vb,
                            ct_sb[i][:, b * NK : (b + 1) * NK],
                            wv_sb[i],
                            start=(i == 0),
                            stop=(i == len(DK_C) - 1),
                        )
                    tf = consts.tile([NK, HEADS * 64], F32, tag=f"vpf{b}")
                    nc.vector.memset(tf, 0.0)
                    tf3 = tf.rearrange("p (h c) -> p h c", c=64)
                    vb3 = vb.rearrange("p (h c) -> p h c", c=DH)
                    nc.vector.tensor_copy(out=tf3[:, :, 0:DH], in_=vb3)
                    nc.vector.memset(tf3[:, :, DH : DH + 1], 1.0)
                    t = consts.tile([NK, HEADS * 64], BF16, tag=f"vp{b}")
                    nc.vector.tensor_copy(out=t, in_=tf)
                    vp_sb.append(t)

                # ---- pipeline stages ----
                # ---- pipeline stage units, each consuming products made a
                # chunk earlier so PE matmuls are (transitively) wait-free.
                # Units of different stages are interleaved round-robin so
                # ACT/DVE evacuations always run under unrelated PE work. ----
                states = [None] * NCHUNKS

                def stage_x(ci):
                    # prefetch x chunk
                    n0 = ci * CHUNK
                    xts = []
                    for i, (d0, dk) in enumerate(DK_Q):
                        t = xt_pool.tile([dk, CHUNK], BF16, tag=f"xt{i}")
                        nc.sync.dma_start(out=t, in_=xT[d0 : d0 + dk, n0 : n0 + CHUNK])
                        xts.append(t)
                    states[ci] = {
                        "xt": xts,
                        "n0": n0,
                        "b": ci // CHUNKS_PER_BATCH,
                        "qt": [None] * NPAIR,
                        "ex": [None] * (2 * NPAIR),
                        "oh": [None] * NPAIR,
                        "st": [None] * NPAIR,
                    }

                def unit_q(ci, p):
                    # q projection for one pair: [104 | CHUNK], heads 0/64
                    state = states[ci]
                    qp = ps_q.tile([104, CHUNK], F32, tag="q")
                    for i in range(len(DK_Q)):
                        nc.tensor.matmul(
                            qp,
                            wq_sb[i][:, p, :],
                            state["xt"][i],
                            start=(i == 0),
                            stop=(i == len(DK_Q) - 1),
                        )
                    qt = qt_pool.tile([104, CHUNK], BF16, tag=f"qt{p}")
                    if p < 2:
                        nc.scalar.copy(out=qt, in_=qp)
                    else:
                        nc.vector.tensor_copy(out=qt, in_=qp)
                    state["qt"][p] = qt

                def unit_s(ci, h):
                    # scores + exp for one head
                    state = states[ci]
                    p, half = h // 2, h % 2
                    hb = 64 * half
                    bs = state["b"] * NK
                    sc = ps_sc.tile([NK, CHUNK], F32, tag="sc")
                    nc.tensor.matmul(
                        sc,
                        kt_sb[p][hb : hb + DH, bs : bs + NK],
                        state["qt"][p][hb : hb + DH, :],
                        start=True,
                        stop=True,
                    )
                    ex = ex_pool.tile([NK, CHUNK], BF16, tag=f"ex{h}")
                    nc.scalar.activation(
                        out=ex, in_=sc, func=mybir.ActivationFunctionType.Exp
                    )
                    state["ex"][h] = ex

                def unit_v(ci, p):
                    # attention-value matmuls + pair evacuation + Z gather
                    state = states[ci]
                    b = state["b"]
                    if p == 0:
                        zgt = zg_pool.tile([8, CHUNK], F32, tag="zg")
                        state["zg"] = zgt
                    zgt = state["zg"]
                    p1 = ps_p1.tile([128, CHUNK], F32, tag="p1")
                    for half in range(2):
                        nc.tensor.matmul(
                            p1[64 * half : 64 * half + 64, :],
                            vp_sb[b][
                                :, (2 * p + half) * 64 : (2 * p + half) * 64 + 64
                            ],
                            state["ex"][2 * p + half],
                            start=True,
                            stop=True,
                        )
                    # single evacuation of the pair tile (Z rows included)
                    oh = oh_pool.tile([128, CHUNK], F32, tag=f"oh{p}")
                    if p < 2:
                        nc.scalar.copy(out=oh, in_=p1)
                    else:
                        nc.vector.tensor_copy(out=oh, in_=p1)
                    state["oh"][p] = oh
                    # gather the pair's Z rows into the chunk tile
                    nc.sync.dma_start(
                        out=zgt[2 * p : 2 * p + 1, :], in_=oh[DH : DH + 1, :]
                    )
                    nc.sync.dma_start(
                        out=zgt[2 * p + 1 : 2 * p + 2, :],
                        in_=oh[64 + DH : 64 + DH + 1, :],
                    )
                    if p == NPAIR - 1:
                        # 1/Z on ACT: exp(-ln Z); Ln/Exp share one act table
                        lg = zf_pool.tile([8, CHUNK], F32, tag="lg")
                        nc.scalar.activation(
                            out=lg, in_=zgt, func=mybir.ActivationFunctionType.Ln
                        )
                        zr = zr_pool.tile([8, CHUNK], F32R, tag="zr")
                        with nc.allow_low_precision(
                            reason="act-table 1/Z well within tolerance"
                        ):
                            nc.scalar.activation(
                                out=zr,
                                in_=lg,
                                func=mybir.ActivationFunctionType.Exp,
                                scale=-1.0,
                            )
                        state["zr"] = zr

                def unit_b(ci, p):
                    # broadcast 1/Z into psum and normalize: st = oh * rb
                    state = states[ci]
                    rb = ps_rb.tile([128, CHUNK], F32, tag="rb")
                    nc.tensor.matmul(rb, sel_sb[p], state["zr"], start=True, stop=True)
                    st = st_pool.tile([128, CHUNK], BF16, tag=f"st{p}")
                    with nc.allow_low_precision(
                        reason="bf16 attention weights within tolerance"
                    ):
                        nc.vector.tensor_mul(st, state["oh"][p], rb)
                    state["st"][p] = st

                def unit_p(ci, j):
                    # out projection for one j block of output channels
                    state = states[ci]
                    j0, jw = JT[j]
                    po = ps_po.tile([128, CHUNK], F32, tag="po")
                    for p in range(NPAIR):
                        nc.tensor.matmul(
                            po[0:jw, :],
                            wo_sb[p][:, j0 : j0 + jw],
                            state["st"][p],
                            start=(p == 0),
                            stop=(p == NPAIR - 1),
                        )
                    oo = oo_pool.tile([jw, CHUNK], BF16, tag="oo")
                    with nc.allow_low_precision(
                        reason="bf16 output well within tolerance"
                    ):
                        nc.vector.tensor_scalar_add(
                            out=oo, in0=po[0:jw, :], scalar1=bo_sb[j]
                        )
                    n0 = state["n0"]
                    nc.sync.dma_start(
                        out=outT[j0 : j0 + jw, n0 : n0 + CHUNK], in_=oo
                    )

                # round-robin unit schedule: between any two units of one
                # stage there are several units of other stages, so each
                # cross-engine dependency has ~a microsecond of PE work in
                # front of it
                SCHED = [
                    ("q", 0), ("s", 0), ("s", 1), ("v", 0), ("b", 0), ("p", 0),
                    ("q", 1), ("s", 2), ("s", 3), ("v", 1), ("b", 1), ("p", 1),
                    ("q", 2), ("s", 4), ("s", 5), ("v", 2), ("b", 2), ("p", 2),
                    ("q", 3), ("s", 6), ("s", 7), ("v", 3), ("b", 3),
                ]
                UNIT = {"q": unit_q, "s": unit_s, "v": unit_v, "b": unit_b,
                        "p": unit_p}
                OFF = {"q": 0, "s": 1, "v": 2, "b": 3, "p": 4}

                # ---- main loop: stages offset so inputs are a chunk old ----
                stage_x(0)
                for ci in range(NCHUNKS + 4):
                    if ci + 1 < NCHUNKS:
                        stage_x(ci + 1)
                    for kind, idx in SCHED:
                        c = ci - OFF[kind]
                        if 0 <= c < NCHUNKS:
                            UNIT[kind](c, idx)

    return _split_multi_waits(nc)


_PROGRAM = None


def _get_program():
    global _PROGRAM
    if _PROGRAM is None:
        _PROGRAM = _build_program()
    return _PROGRAM


def _prep_weights(Wq, Wk, Wv, Wo, bo, gamma_q, gamma_k, gamma_v, gamma_out):
    scale = DH ** -0.5
    Wqp = (gamma_q[:, None] * Wq) * scale          # [320i, 320d]
    Wkp = gamma_k[:, None] * Wk                    # [320i, 768d]
    Wvp = gamma_v[:, None] * Wv                    # [320i, 768d]
    Wop = gamma_out[:, None] * Wo                  # [320j, 320i]
    bop = (gamma_out * bo).astype(np.float32)[:, None]

    wq_dev = np.zeros((QD, NPAIR, 104), np.float32)
    wk_dev = np.zeros((CD, NPAIR, 104), np.float32)
    for p in range(NPAIR):
        hA, hB = 2 * p, 2 * p + 1
        wq_dev[:, p, 0:DH] = Wqp[hA * DH : (hA + 1) * DH, :].T
        wq_dev[:, p, 64 : 64 + DH] = Wqp[hB * DH : (hB + 1) * DH, :].T
        wk_dev[:, p, 0:DH] = Wkp[hA * DH : (hA + 1) * DH, :].T
        wk_dev[:, p, 64 : 64 + DH] = Wkp[hB * DH : (hB + 1) * DH, :].T
    wv_dev = np.ascontiguousarray(Wvp.T, dtype=np.float32)     # [768, 320]
    # st rows per pair: 0..39 = head A channels, 40 = junk (Z/Z), 64..103 =
    # head B channels, 104 = junk; the rest is zero
    wo_dev = np.zeros((NPAIR, 128, QD), np.float32)
    for p in range(NPAIR):
        hA, hB = 2 * p, 2 * p + 1
        wo_dev[p, 0:DH, :] = Wop[:, hA * DH : (hA + 1) * DH].T
        wo_dev[p, 64 : 64 + DH, :] = Wop[:, hB * DH : (hB + 1) * DH].T
    sel_dev = np.zeros((8, NPAIR, 128), np.float32)
    for p in range(NPAIR):
        sel_dev[2 * p, p, 0:64] = 1.0
        sel_dev[2 * p + 1, p, 64:128] = 1.0
    return wq_dev, wk_dev, wv_dev, wo_dev, bop, sel_dev


def kernel(x, context, Wq, Wk, Wv, Wo, bo, gamma_q, gamma_k, gamma_v, gamma_out):
    global LAST_EXEC_NS, LAST_RESULTS
    x = np.asarray(x, np.float32)
    context = np.asarray(context, np.float32)
    wq_dev, wk_dev, wv_dev, wo_dev, bop, sel_dev = _prep_weights(
        np.asarray(Wq, np.float32), np.asarray(Wk, np.float32),
        np.asarray(Wv, np.float32), np.asarray(Wo, np.float32),
        np.asarray(bo, np.float32), np.asarray(gamma_q, np.float32),
        np.asarray(gamma_k, np.float32), np.asarray(gamma_v, np.float32),
        np.asarray(gamma_out, np.float32),
    )

    in_maps = []
    for c in range(NCORES):
        xs = x[c * BL : (c + 1) * BL].reshape(NLOC, QD)
        cs = context[c * BL : (c + 1) * BL].reshape(NKL, CD)
        in_maps.append(
            {
                "xT": np.ascontiguousarray(xs.T).astype(BF16NP),
                "cT": np.ascontiguousarray(cs.T).astype(BF16NP),
                "wq": wq_dev.astype(BF16NP),
                "wk": wk_dev.astype(BF16NP),
                "wv": wv_dev.astype(BF16NP),
                "wo": wo_dev.astype(BF16NP),
                "bo": bop,
                "sel": sel_dev,
            }
        )

    nc = _get_program()
    res = run_bass_kernel_spmd(nc, in_maps, list(range(NCORES)))
    LAST_EXEC_NS = res.exec_time_ns
    LAST_RESULTS = res

    out = np.empty((B, NQ, QD), np.float32)
    for c in range(NCORES):
        out[c * BL : (c + 1) * BL] = (
            np.asarray(res.results[c]["outT"]).astype(np.float32).T.reshape(BL, NQ, QD)
        )
    return out


# revision 32
# speedup vs baseline: 1.6426x; 1.1636x over previous
"""Cross-attention kernel for Trainium2 (8 NeuronCores, data-parallel over batch).

Reference computation (per batch b):
    q = (x @ Wq.T) * gamma_q ; k = (ctx @ Wk.T) * gamma_k ; v = (ctx @ Wv.T) * gamma_v
    per head: o = softmax(q k^T / sqrt(dh)) v
    out = (concat_heads(o) @ Wo.T + bo) * gamma_out

Device strategy (per core, 4 batches, n = 4*4096 = 16384 query rows):
  - Transposed world: activations live as [channel | n]; contraction dims sit
    on partitions and softmax sums come out of the PE via a ones-column in V.
  - Heads are packed in PAIRS at partition bases {0, 64}. The two AV matmuls
    of a pair write ONE [128|512] psum tile directly (head B lands at
    partitions 64-127 via matmul tile_position=(0,64)), so no shift-DMAs.
  - Z rows (partition 40 / 104 of the pair psum) reach SBUF through the
    single pair evacuation; tiny SBUF->SBUF DMAs gather them into [4|512]
    tiles, a fast-approx DVE reciprocal per 2-pair group, then a K=4
    selector matmul broadcasts 1/Z across the 64-row halves into psum.
  - Two-deep software pipeline to keep the PE HAM-warm: chunk c emits
    q-projection(c), broadcast+normalize(c-1), out-projection(c-2), then
    attention(c). Every matmul's inputs are ready ~a chunk before it issues.
  - Output stored bf16 (halves output DMA traffic; well within tolerance).
"""

import os
import sys

import ml_dtypes
import numpy as np

BF16NP = ml_dtypes.bfloat16

for _p in ("/opt/trn_rl_repo",):
    if _p not in sys.path and os.path.isdir(_p):
        sys.path.append(_p)

import concourse.bass as bass
import concourse.mybir as mybir
import concourse.tile as tile
from concourse.bass_utils import run_bass_kernel_spmd

HEADS = 8
DH = 40
QD = 320            # query/input channel dim == inner dim
CD = 768            # context channel dim
B, NQ, NK = 32, 4096, 77
NCORES = 8
BL = B // NCORES    # batches per core = 4
NLOC = BL * NQ      # query rows per core = 16384
NKL = BL * NK       # context rows per core = 308
CHUNK = 512
NCHUNKS = NLOC // CHUNK          # 32
CHUNKS_PER_BATCH = NQ // CHUNK   # 8
NPAIR = HEADS // 2               # 4 head pairs; pair p = heads (2p, 2p+1)

F32 = mybir.dt.float32
F32R = mybir.dt.float32r
BF16 = mybir.dt.bfloat16

# K-chunking of the contraction dims
DK_Q = [(0, 128), (128, 128), (256, 64)]                       # QD = 320
DK_C = [(i * 128, 128) for i in range(6)]                      # CD = 768
JT = [(0, 128), (128, 128), (256, 64)]                         # out channels 320

LAST_EXEC_NS = None
LAST_RESULTS = None


def _split_multi_waits(nc):
    """Walrus codegen allows at most ONE semaphore wait per instruction.
    Split any instruction with N>1 waits into (N-1) same-engine NoOps, each
    carrying one wait, followed by the original instruction with the last
    wait. Engines execute their streams in order, so this is equivalent."""
    k = 0
    for blk in nc.m.functions[0].blocks:
        insts = list(blk.instructions)
        out = []
        for ins in insts:
            si = getattr(ins, "sync_info", None)
            if si is not None and len(si.on_wait) > 1:
                waits = list(si.on_wait)
                for w in waits[:-1]:
                    nop = mybir.InstNoOp(name=f"wsplit-{k}")
                    k += 1
                    nop.engine = ins.engine
                    nop.sync_info = mybir.SyncInfo(on_wait=[w], on_update=[])
                    out.append(nop)
                ins.sync_info = mybir.SyncInfo(
                    on_wait=[waits[-1]], on_update=list(si.on_update)
                )
            out.append(ins)
        if len(out) != len(insts):
            blk.instructions = out
    return nc


def _build_program():
    nc = bass.Bass(trn_type="TRN2")

    xT = nc.declare_dram_parameter("xT", [QD, NLOC], BF16, isOutput=False)
    cT = nc.declare_dram_parameter("cT", [CD, NKL], BF16, isOutput=False)
    wq = nc.declare_dram_parameter("wq", [QD, NPAIR, 104], BF16, isOutput=False)
    wk = nc.declare_dram_parameter("wk", [CD, NPAIR, 104], BF16, isOutput=False)
    wv = nc.declare_dram_parameter("wv", [CD, QD], BF16, isOutput=False)
    wo = nc.declare_dram_parameter("wo", [NPAIR, 128, QD], BF16, isOutput=False)
    bo = nc.declare_dram_parameter("bo", [QD, 1], F32, isOutput=False)
    sel = nc.declare_dram_parameter("sel", [8, NPAIR, 128], F32R, isOutput=False)
    outT = nc.declare_dram_parameter("outT", [QD, NLOC], BF16, isOutput=True)

    with tile.TileContext(nc) as tc:
        with (
            tc.tile_pool(name="consts", bufs=1) as consts,
            tc.tile_pool(name="xt", bufs=3) as xt_pool,
            tc.tile_pool(name="qt", bufs=3) as qt_pool,
            tc.tile_pool(name="ex", bufs=3) as ex_pool,
            tc.tile_pool(name="oh", bufs=4) as oh_pool,
            tc.tile_pool(name="zg", bufs=2) as zg_pool,
            tc.tile_pool(name="zf", bufs=2) as zf_pool,
            tc.tile_pool(name="zr", bufs=4) as zr_pool,
            tc.tile_pool(name="st", bufs=3) as st_pool,
            tc.tile_pool(name="oo", bufs=3) as oo_pool,
        ):
            # ---- load + stage constants ----
            def staged(shape, dtype, tag, src):
                s = consts.tile(shape, dtype, tag=f"s{tag}")
                nc.sync.dma_start(out=s, in_=src)
                t = consts.tile(shape, dtype, tag=tag)
                nc.vector.tensor_copy(out=t, in_=s)
                return t

            wq_sb = [
                staged([dk, NPAIR, 104], BF16, f"wq{i}", wq[d0 : d0 + dk, :, :])
                for i, (d0, dk) in enumerate(DK_Q)
            ]
            wo_sb = [
                staged([128, QD], BF16, f"wo{p}", wo[p, :, :]) for p in range(NPAIR)
            ]
            wk_sb = [
                staged([dk, NPAIR, 104], BF16, f"wk{i}", wk[d0 : d0 + dk, :, :])
                for i, (d0, dk) in enumerate(DK_C)
            ]
            wv_sb = [
                staged([dk, QD], BF16, f"wv{i}", wv[d0 : d0 + dk, :])
                for i, (d0, dk) in enumerate(DK_C)
            ]
            ct_sb = [
                staged([dk, NKL], BF16, f"ct{i}", cT[d0 : d0 + dk, :])
                for i, (d0, dk) in enumerate(DK_C)
            ]
            bo_sb = []
            for j, (j0, jw) in enumerate(JT):
                t = consts.tile([jw, 1], F32, tag=f"bo{j}")
                nc.sync.dma_start(out=t, in_=bo[j0 : j0 + jw, :])
                bo_sb.append(t)

            # selector matrices for the 1/Z broadcast matmul: sel_t[:, p, :]
            # is [8|128]; out rows 0-63 copy zr row 2p, rows 64-127 row 2p+1
            sel_t = consts.tile([8, NPAIR, 128], F32R, tag="sel")
            nc.sync.dma_start(out=sel_t, in_=sel[:, :, :])
            sel_sb = [sel_t[:, p, :] for p in range(NPAIR)]

            with (
                tc.tile_pool(name="ps_q", bufs=2, space="PSUM") as ps_q,
                tc.tile_pool(name="ps_sc", bufs=2, space="PSUM") as ps_sc,
                tc.tile_pool(name="ps_p1", bufs=2, space="PSUM") as ps_p1,
                tc.tile_pool(name="ps_rb", bufs=1, space="PSUM") as ps_rb,
                tc.tile_pool(name="ps_po", bufs=1, space="PSUM") as ps_po,
            ):
                # ---- setup projections (psum via the main pools) ----
                kt_sb = []
                vp_sb = []
                # kT[p]: [104 | NKL], heads of pair p at partitions 0 / 64
                for p in range(NPAIR):
                    kp = ps_q.tile([104, NKL], F32, tag="q")
                    for i in range(len(DK_C)):
                        nc.tensor.matmul(
                            kp,
                            wk_sb[i][:, p, :],
                            ct_sb[i],
                            start=(i == 0),
                            stop=(i == len(DK_C) - 1),
                        )
                    t = consts.tile([104, NKL], BF16, tag=f"kt{p}")
                    nc.scalar.copy(out=t, in_=kp)
                    kt_sb.append(t)

                # vp[b]: [77 | 8*64]; head h: cols 64h..64h+39 = v channels,
                # col 64h+40 = 1 (Z), rest 0
                for b in range(BL):
                    vb = ps_p1.tile([NK, QD], F32, tag="p1")
                    for i in range(len(DK_C)):
                        nc.tensor.matmul(
                            vb,
                            ct_sb[i][:, b * NK : (b + 1) * NK],
                            wv_sb[i],
                            start=(i == 0),
                            stop=(i == len(DK_C) - 1),
                        )
                    tf = consts.tile([NK, HEADS * 64], F32, tag=f"vpf{b}")
                    nc.vector.memset(tf, 0.0)
                    tf3 = tf.rearrange("p (h c) -> p h c", c=64)
                    vb3 = vb.rearrange("p (h c) -> p h c", c=DH)
                    nc.vector.tensor_copy(out=tf3[:, :, 0:DH], in_=vb3)
                    nc.vector.memset(tf3[:, :, DH : DH + 1], 1.0)
                    t = consts.tile([NK, HEADS * 64], BF16, tag=f"vp{b}")
                    nc.vector.tensor_copy(out=t, in_=tf)
                    vp_sb.append(t)

                # ---- pipeline stages ----
                # ---- pipeline stage units, each consuming products made a
                # chunk earlier so PE matmuls are (transitively) wait-free.
                # Units of different stages are interleaved round-robin so
                # ACT/DVE evacuations always run under unrelated PE work. ----
                states = [None] * NCHUNKS

                def stage_x(ci):
                    # prefetch x chunk
                    n0 = ci * CHUNK
                    xts = []
                    for i, (d0, dk) in enumerate(DK_Q):
                        t = xt_pool.tile([dk, CHUNK], BF16, tag=f"xt{i}")
                        nc.sync.dma_start(out=t, in_=xT[d0 : d0 + dk, n0 : n0 + CHUNK])
                        xts.append(t)
                    states[ci] = {
                        "xt": xts,
                        "n0": n0,
                        "b": ci // CHUNKS_PER_BATCH,
                        "qt": [None] * NPAIR,
                        "ex": [None] * (2 * NPAIR),
                        "oh": [None] * NPAIR,
                        "st": [None] * NPAIR,
                    }

                def unit_q(ci, p):
                    # q projection for one pair: [104 | CHUNK], heads 0/64
                    state = states[ci]
                    qp = ps_q.tile([104, CHUNK], F32, tag="q")
                    for i in range(len(DK_Q)):
                        nc.tensor.matmul(
                            qp,
                            wq_sb[i][:, p, :],
                            state["xt"][i],
                            start=(i == 0),
                            stop=(i == len(DK_Q) - 1),
                        )
                    qt = qt_pool.tile([104, CHUNK], BF16, tag=f"qt{p}")
                    if p < 2:
                        nc.scalar.copy(out=qt, in_=qp)
                    else:
                        nc.vector.tensor_copy(out=qt, in_=qp)
                    state["qt"][p] = qt

                def unit_s(ci, h):
                    # scores + exp for one head
                    state = states[ci]
                    p, half = h // 2, h % 2
                    hb = 64 * half
                    bs = state["b"] * NK
                    sc = ps_sc.tile([NK, CHUNK], F32, tag="sc")
                    nc.tensor.matmul(
                        sc,
                        kt_sb[p][hb : hb + DH, bs : bs + NK],
                        state["qt"][p][hb : hb + DH, :],
                        start=True,
                        stop=True,
                    )
                    ex = ex_pool.tile([NK, CHUNK], BF16, tag=f"ex{h}")
                    nc.scalar.activation(
                        out=ex, in_=sc, func=mybir.ActivationFunctionType.Exp
                    )
                    state["ex"][h] = ex

                def unit_v(ci, p):
                    # attention-value matmuls + pair evacuation + Z gather
                    state = states[ci]
                    b = state["b"]
                    if p == 0:
                        zgt = zg_pool.tile([8, CHUNK], F32, tag="zg")
                        state["zg"] = zgt
                    zgt = state["zg"]
                    p1 = ps_p1.tile([128, CHUNK], F32, tag="p1")
                    for half in range(2):
                        nc.tensor.matmul(
                            p1[64 * half : 64 * half + 64, :],
                            vp_sb[b][
                                :, (2 * p + half) * 64 : (2 * p + half) * 64 + 64
                            ],
                            state["ex"][2 * p + half],
                            start=True,
                            stop=True,
                        )
                    # single evacuation of the pair tile (Z rows included)
                    oh = oh_pool.tile([128, CHUNK], F32, tag=f"oh{p}")
                    if p < 2:
                        nc.scalar.copy(out=oh, in_=p1)
                    else:
                        nc.vector.tensor_copy(out=oh, in_=p1)
                    state["oh"][p] = oh
                    # gather the pair's Z rows into the chunk tile
                    nc.sync.dma_start(
                        out=zgt[2 * p : 2 * p + 1, :], in_=oh[DH : DH + 1, :]
                    )
                    nc.sync.dma_start(
                        out=zgt[2 * p + 1 : 2 * p + 2, :],
                        in_=oh[64 + DH : 64 + DH + 1, :],
                    )
                    if p == NPAIR - 1:
                        # 1/Z on ACT: exp(-ln Z); Ln/Exp share one act table
                        lg = zf_pool.tile([8, CHUNK], F32, tag="lg")
                        nc.scalar.activation(
                            out=lg, in_=zgt, func=mybir.ActivationFunctionType.Ln
                        )
                        zr = zr_pool.tile([8, CHUNK], F32R, tag="zr")
                        with nc.allow_low_precision(
                            reason="act-table 1/Z well within tolerance"
                        ):
                            nc.scalar.activation(
                                out=zr,
                                in_=lg,
                                func=mybir.ActivationFunctionType.Exp,
                                scale=-1.0,
                            )
                        state["zr"] = zr

                def unit_b(ci, p):
                    # broadcast 1/Z into psum and normalize: st = oh * rb
                    state = states[ci]
                    rb = ps_rb.tile([128, CHUNK], F32, tag="rb")
                    nc.tensor.matmul(rb, sel_sb[p], state["zr"], start=True, stop=True)
                    st = st_pool.tile([128, CHUNK], BF16, tag=f"st{p}")
                    with nc.allow_low_precision(
                        reason="bf16 attention weights within tolerance"
                    ):
                        nc.vector.tensor_mul(st, state["oh"][p], rb)
                    state["st"][p] = st

                def unit_p(ci, j):
                    # out projection for one j block of output channels
                    state = states[ci]
                    j0, jw = JT[j]
                    po = ps_po.tile([128, CHUNK], F32, tag="po")
                    for p in range(NPAIR):
                        nc.tensor.matmul(
                            po[0:jw, :],
                            wo_sb[p][:, j0 : j0 + jw],
                            state["st"][p],
                            start=(p == 0),
                            stop=(p == NPAIR - 1),
                        )
                    oo = oo_pool.tile([jw, CHUNK], BF16, tag="oo")
                    with nc.allow_low_precision(
                        reason="bf16 output well within tolerance"
                    ):
                        nc.vector.tensor_scalar_add(
                            out=oo, in0=po[0:jw, :], scalar1=bo_sb[j]
                        )
                    n0 = state["n0"]
                    nc.sync.dma_start(
                        out=outT[j0 : j0 + jw, n0 : n0 + CHUNK], in_=oo
                    )

                # round-robin unit schedule: between any two units of one
                # stage there are several units of other stages, so each
                # cross-engine dependency has ~a microsecond of PE work in
                # front of it
                SCHED = [
                    ("q", 0), ("s", 0), ("s", 1), ("v", 0), ("b", 0), ("p", 0),
                    ("q", 1), ("s", 2), ("s", 3), ("v", 1), ("b", 1), ("p", 1),
                    ("q", 2), ("s", 4), ("s", 5), ("v", 2), ("b", 2), ("p", 2),
                    ("q", 3), ("s", 6), ("s", 7), ("v", 3), ("b", 3),
                ]
                UNIT = {"q": unit_q, "s": unit_s, "v": unit_v, "b": unit_b,
                        "p": unit_p}
                OFF = {"q": 0, "s": 1, "v": 2, "b": 4, "p": 5}

                # ---- main loop: stages offset so inputs are a chunk old ----
                stage_x(0)
                for ci in range(NCHUNKS + 5):
                    if ci + 1 < NCHUNKS:
                        stage_x(ci + 1)
                    for kind, idx in SCHED:
                        c = ci - OFF[kind]
                        if 0 <= c < NCHUNKS:
                            UNIT[kind](c, idx)

    return _split_multi_waits(nc)


_PROGRAM = None


def _get_program():
    global _PROGRAM
    if _PROGRAM is None:
        _PROGRAM = _build_program()
    return _PROGRAM


def _prep_weights(Wq, Wk, Wv, Wo, bo, gamma_q, gamma_k, gamma_v, gamma_out):
    scale = DH ** -0.5
    Wqp = (gamma_q[:, None] * Wq) * scale          # [320i, 320d]
    Wkp = gamma_k[:, None] * Wk                    # [320i, 768d]
    Wvp = gamma_v[:, None] * Wv                    # [320i, 768d]
    Wop = gamma_out[:, None] * Wo                  # [320j, 320i]
    bop = (gamma_out * bo).astype(np.float32)[:, None]

    wq_dev = np.zeros((QD, NPAIR, 104), np.float32)
    wk_dev = np.zeros((CD, NPAIR, 104), np.float32)
    for p in range(NPAIR):
        hA, hB = 2 * p, 2 * p + 1
        wq_dev[:, p, 0:DH] = Wqp[hA * DH : (hA + 1) * DH, :].T
        wq_dev[:, p, 64 : 64 + DH] = Wqp[hB * DH : (hB + 1) * DH, :].T
        wk_dev[:, p, 0:DH] = Wkp[hA * DH : (hA + 1) * DH, :].T
        wk_dev[:, p, 64 : 64 + DH] = Wkp[hB * DH : (hB + 1) * DH, :].T
    wv_dev = np.ascontiguousarray(Wvp.T, dtype=np.float32)     # [768, 320]
    # st rows per pair: 0..39 = head A channels, 40 = junk (Z/Z), 64..103 =
    # head B channels, 104 = junk; the rest is zero
    wo_dev = np.zeros((NPAIR, 128, QD), np.float32)
    for p in range(NPAIR):
        hA, hB = 2 * p, 2 * p + 1
        wo_dev[p, 0:DH, :] = Wop[:, hA * DH : (hA + 1) * DH].T
        wo_dev[p, 64 : 64 + DH, :] = Wop[:, hB * DH : (hB + 1) * DH].T
    sel_dev = np.zeros((8, NPAIR, 128), np.float32)
    for p in range(NPAIR):
        sel_dev[2 * p, p, 0:64] = 1.0
        sel_dev[2 * p + 1, p, 64:128] = 1.0
    return wq_dev, wk_dev, wv_dev, wo_dev, bop, sel_dev


def kernel(x, context, Wq, Wk, Wv, Wo, bo, gamma_q, gamma_k, gamma_v, gamma_out):
    global LAST_EXEC_NS, LAST_RESULTS
    x = np.asarray(x, np.float32)
    context = np.asarray(context, np.float32)
    wq_dev, wk_dev, wv_dev, wo_dev, bop, sel_dev = _prep_weights(
        np.asarray(Wq, np.float32), np.asarray(Wk, np.float32),
        np.asarray(Wv, np.float32), np.asarray(Wo, np.float32),
        np.asarray(bo, np.float32), np.asarray(gamma_q, np.float32),
        np.asarray(gamma_k, np.float32), np.asarray(gamma_v, np.float32),
        np.asarray(gamma_out, np.float32),
    )

    in_maps = []
    for c in range(NCORES):
        xs = x[c * BL : (c + 1) * BL].reshape(NLOC, QD)
        cs = context[c * BL : (c + 1) * BL].reshape(NKL, CD)
        in_maps.append(
            {
                "xT": np.ascontiguousarray(xs.T).astype(BF16NP),
                "cT": np.ascontiguousarray(cs.T).astype(BF16NP),
                "wq": wq_dev.astype(BF16NP),
                "wk": wk_dev.astype(BF16NP),
                "wv": wv_dev.astype(BF16NP),
                "wo": wo_dev.astype(BF16NP),
                "bo": bop,
                "sel": sel_dev,
            }
        )

    nc = _get_program()
    res = run_bass_kernel_spmd(nc, in_maps, list(range(NCORES)))
    LAST_EXEC_NS = res.exec_time_ns
    LAST_RESULTS = res

    out = np.empty((B, NQ, QD), np.float32)
    for c in range(NCORES):
        out[c * BL : (c + 1) * BL] = (
            np.asarray(res.results[c]["outT"]).astype(np.float32).T.reshape(BL, NQ, QD)
        )
    return out
